# revision 7
# baseline (speedup 1.0000x reference)
"""Log-space matmul kernel for Trainium2 (8 NeuronCores, SPMD).

Problem: out[n, m] = logsumexp_k(log_A[n, k] + log_B[k, m])
         log_A: [1024, 512] f32, log_B: [512, 1024] f32 -> out [1024, 1024] f32

Reformulation: out = log(exp(log_A) @ exp(log_B)).
Inputs are standard normal (|x| <~ 5.5), so exp() stays comfortably inside
fp32 range without max-shifting; sums over K=512 stay < ~1e8. The fp32
pipeline matches the reference logsumexp to ~5e-7 relative error.

Sharding: 4-way over N rows x 2-way over M cols (8 cores). Each core:
  - loads its A slab [256, 512] and B slab [512, 512]
  - exponentiates both on ScalarE (ACT)
  - transposes exp(A) via TensorE (PE transpose, fp32) to get lhsT tiles
  - matmuls on TensorE, accumulating over K in PSUM (fp32)
  - takes Ln of the PSUM result on ScalarE, DMAs the [256, 512] slab out
"""

import os
from contextlib import ExitStack

import numpy as np

import concourse.bass as bass
import concourse.mybir as mybir
import concourse.tile as tile
from concourse.bass_utils import run_bass_kernel_spmd
from concourse.masks import make_identity

# This walrus build rejects any instruction carrying more than one semaphore
# wait ("Too many sync wait commands"). Tile's kernel-tail drain waits on every
# proc at once; split those waits across single-wait NOPs instead.
_orig_drain_and_barrier = tile.TileContext._drain_and_barrier


def _split_drain_and_barrier(self, tick_clock, wait_clock):
    from concourse.vector_clock import ScopedClock

    probe = self.nc.sync.nop(nofuse=True)
    wait_clock.add_sem_waits(probe.ins, ScopedClock({None: tick_clock.global_clock}))
    si = probe.ins.sync_info
    waits = list(si.on_wait)
    si.on_wait = waits[:1]
    probe.ins.sync_info = si
    for w in waits[1:]:
        nop = self.nc.sync.nop(nofuse=True)
        nop.ins.sync_info = mybir.SyncInfo(on_wait=[w], on_update=[])

    self.nc.sync.drain()
    self.nc.all_engine_barrier()
    assert self.sems is not None
    popped = self.nc._tile_sem_poison_stack.pop()
    assert popped is self._sem_poison
    self.nc.clear_and_free_semaphores(list(self.sems.allocated().values()))
    self.nc.all_engine_barrier()


tile.TileContext._drain_and_barrier = _split_drain_and_barrier

N, K, M = 1024, 512, 1024
GRID_N, GRID_M = 4, 2
SN, SM = N // GRID_N, M // GRID_M  # 256, 512 per-core output slab
P = 128
KT = K // P  # 4 k-tiles
NT = SN // P  # 2 n-tiles per core
F32 = mybir.dt.float32
AF = mybir.ActivationFunctionType


def _build_nc() -> bass.Bass:
    nc = bass.Bass()
    a_in = nc.declare_dram_parameter("a_in", [SN, K], F32, isOutput=False)
    b_in = nc.declare_dram_parameter("b_in", [K, SM], F32, isOutput=False)
    out = nc.declare_dram_parameter("out", [SN, SM], F32, isOutput=True)

    with tile.TileContext(nc) as tc, ExitStack() as ctx:
        pool = ctx.enter_context(tc.tile_pool(name="sbuf", bufs=1))
        tpsum = ctx.enter_context(
            tc.tile_pool(name="tpsum", bufs=2, space=bass.MemorySpace.PSUM)
        )
        opsum = ctx.enter_context(
            tc.tile_pool(name="opsum", bufs=2, space=bass.MemorySpace.PSUM)
        )

        ident = pool.tile([P, P], F32)
        make_identity(nc, ident[:])

        # Walrus rejects a Matmult carrying >1 semaphore wait. Dummy PE
        # transposes absorb cross-engine waits one at a time so every real
        # matmul below needs at most one inline wait.
        dpsum = ctx.enter_context(
            tc.tile_pool(name="dpsum", bufs=1, space=bass.MemorySpace.PSUM)
        )
        dummy = dpsum.tile([P, P], F32)
        # D1: waits only on GPSIMD (ident ready); PE clock then covers Pool.
        nc.tensor.transpose(dummy[:], ident[:], ident[:])

        # ---- A chain: load -> exp -> PE-transpose per (ki, t) -> aT ----
        a_raw = pool.tile([P, NT, K], F32)
        nc.sync.dma_start(a_raw[:], a_in.rearrange("(t p) k -> p t k", p=P))
        a_exp = pool.tile([P, NT, K], F32)
        nc.scalar.activation(a_exp[:], a_raw[:], AF.Exp)

        aT = pool.tile([P, KT, NT, P], F32)  # [k, (ki, t, n)]
        for ki in range(KT):
            for t in range(NT):
                tp = tpsum.tile([P, P], F32)
                nc.tensor.transpose(
                    tp[:], a_exp[:, t, ki * P : (ki + 1) * P], ident[:]
                )
                nc.vector.tensor_copy(aT[:, ki, t, :], tp[:])

        # ---- B chain: load -> exp ----
        b_raw = pool.tile([P, KT, SM], F32)
        nc.sync.dma_start(b_raw[:], b_in.rearrange("(ki p) m -> p ki m", p=P))
        b_exp = pool.tile([P, KT, SM], F32)
        nc.scalar.activation(b_exp[:], b_raw[:], AF.Exp)

        # D2: waits only on DVE (last aT copy); the first real matmul of the
        # accumulation groups then only needs the ACT (b_exp) wait.
        nc.tensor.transpose(dummy[:], aT[:, KT - 1, NT - 1, :], ident[:])

        # ---- matmul: psum[t] += aT[ki,t].T @ b_exp[ki] over ki ----
        out_sb = pool.tile([P, NT, SM], F32)
        for t in range(NT):
            ps = opsum.tile([P, SM], F32)
            for ki in range(KT):
                nc.tensor.matmul(
                    ps[:],
                    aT[:, ki, t, :],
                    b_exp[:, ki, :],
                    start=(ki == 0),
                    stop=(ki == KT - 1),
                )
            nc.scalar.activation(out_sb[:, t, :], ps[:], AF.Ln)
            nc.sync.dma_start(out[t * P : (t + 1) * P, :], out_sb[:, t, :])

    return nc


_NC_CACHE: list = []


def _get_nc() -> bass.Bass:
    if not _NC_CACHE:
        _NC_CACHE.append(_build_nc())
    return _NC_CACHE[0]


def kernel(log_A: np.ndarray, log_B: np.ndarray) -> np.ndarray:
    log_A = np.ascontiguousarray(np.asarray(log_A, dtype=np.float32))
    log_B = np.ascontiguousarray(np.asarray(log_B, dtype=np.float32))
    assert log_A.shape == (N, K) and log_B.shape == (K, M)

    in_maps = []
    for c in range(GRID_N * GRID_M):
        i, j = divmod(c, GRID_M)
        in_maps.append(
            {
                "a_in": np.ascontiguousarray(log_A[i * SN : (i + 1) * SN, :]),
                "b_in": np.ascontiguousarray(log_B[:, j * SM : (j + 1) * SM]),
            }
        )

    nc = _get_nc()
    trace = bool(int(os.environ.get("KERNEL_TRACE", "0")))
    res = run_bass_kernel_spmd(
        nc,
        in_maps,
        list(range(GRID_N * GRID_M)),
        trace=trace,
        tmpdir=globals().get("_TRACE_TMPDIR") if trace else None,
    )

    out = np.empty((N, M), dtype=np.float32)
    for c, r in enumerate(res.results):
        i, j = divmod(c, GRID_M)
        out[i * SN : (i + 1) * SN, j * SM : (j + 1) * SM] = r["out"]
    # stash for test harness introspection
    kernel.last_results = res
    return out


# revision 11
# speedup vs baseline: 1.3394x; 1.3394x over previous
"""Log-space matmul kernel for Trainium2 (8 NeuronCores, SPMD).

Problem: out[n, m] = logsumexp_k(log_A[n, k] + log_B[k, m])
         log_A: [1024, 512] f32, log_B: [512, 1024] f32 -> out [1024, 1024] f32

Reformulation: out = log(exp(log_A) @ exp(log_B)).
Inputs are standard normal (|x| <~ 5.5), so exp() stays comfortably inside
fp32 range without max-shifting; sums over K=512 stay < ~1e8. The fp32
pipeline matches the reference logsumexp to ~5e-7 relative error.

Sharding: 4-way over N rows x 2-way over M cols (8 cores). Each core:
  - loads its A slab [256, 512] and B slab [512, 512]
  - exponentiates both on ScalarE (ACT)
  - transposes exp(A) via TensorE (PE transpose, fp32) to get lhsT tiles
  - matmuls on TensorE, accumulating over K in PSUM (fp32)
  - takes Ln of the PSUM result on ScalarE, DMAs the [256, 512] slab out
"""

import os
from contextlib import ExitStack

import numpy as np

import concourse.bass as bass
import concourse.mybir as mybir
import concourse.tile as tile
from concourse.bass_utils import run_bass_kernel_spmd
from concourse.masks import make_identity

# This walrus build rejects any instruction carrying more than one semaphore
# wait ("Too many sync wait commands"). Tile's kernel-tail drain waits on every
# proc at once; split those waits across single-wait NOPs instead.
_orig_drain_and_barrier = tile.TileContext._drain_and_barrier


def _split_drain_and_barrier(self, tick_clock, wait_clock):
    from concourse.vector_clock import ScopedClock

    probe = self.nc.sync.nop(nofuse=True)
    wait_clock.add_sem_waits(probe.ins, ScopedClock({None: tick_clock.global_clock}))
    si = probe.ins.sync_info
    waits = list(si.on_wait)
    si.on_wait = waits[:1]
    probe.ins.sync_info = si
    for w in waits[1:]:
        nop = self.nc.sync.nop(nofuse=True)
        nop.ins.sync_info = mybir.SyncInfo(on_wait=[w], on_update=[])

    self.nc.sync.drain()
    self.nc.all_engine_barrier()
    assert self.sems is not None
    popped = self.nc._tile_sem_poison_stack.pop()
    assert popped is self._sem_poison
    self.nc.clear_and_free_semaphores(list(self.sems.allocated().values()))
    self.nc.all_engine_barrier()


tile.TileContext._drain_and_barrier = _split_drain_and_barrier

N, K, M = 1024, 512, 1024
GRID_N, GRID_M = 4, 2
SN, SM = N // GRID_N, M // GRID_M  # 256, 512 per-core output slab
P = 128
KT = K // P  # 4 k-tiles
NT = SN // P  # 2 n-tiles per core
F32 = mybir.dt.float32
AF = mybir.ActivationFunctionType


BF16 = mybir.dt.bfloat16
B_CHUNKS = 2
KC = KT // B_CHUNKS  # k-tiles per b chunk


def _build_nc() -> bass.Bass:
    nc = bass.Bass()
    a_in = nc.declare_dram_parameter("a_in", [SN, K], F32, isOutput=False)
    b_in = nc.declare_dram_parameter("b_in", [K, SM], F32, isOutput=False)
    out = nc.declare_dram_parameter("out", [SN, SM], F32, isOutput=True)

    with tile.TileContext(nc) as tc, ExitStack() as ctx:
        pool = ctx.enter_context(tc.tile_pool(name="sbuf", bufs=1))
        tpsum = ctx.enter_context(
            tc.tile_pool(name="tpsum", bufs=4, space=bass.MemorySpace.PSUM)
        )
        opsum = ctx.enter_context(
            tc.tile_pool(name="opsum", bufs=1, space=bass.MemorySpace.PSUM)
        )
        dpsum = ctx.enter_context(
            tc.tile_pool(name="dpsum", bufs=1, space=bass.MemorySpace.PSUM)
        )

        # ---- input DMAs first: b on the ACT HWDGE ring (2 chunks), a on SP ----
        b_raw = pool.tile([P, KT, SM], F32)
        b_view = b_in.rearrange("(ki p) m -> p ki m", p=P)
        for c in range(B_CHUNKS):
            nc.scalar.dma_start(
                b_raw[:, c * KC : (c + 1) * KC, :],
                b_view[:, c * KC : (c + 1) * KC, :],
            )
        a_raw = pool.tile([P, NT, K], F32)
        nc.sync.dma_start(a_raw[:], a_in.rearrange("(t p) k -> p t k", p=P))

        ident = pool.tile([P, P], BF16)
        make_identity(nc, ident[:])

        # Walrus rejects any Matmult carrying >1 semaphore wait. Dummy PE
        # transposes absorb cross-engine waits one at a time so every real
        # matmul below needs at most one inline wait.
        dummy = dpsum.tile([P, P], BF16)
        # D1: waits only on GPSIMD (ident ready); PE clock then covers Pool.
        nc.tensor.transpose(dummy[:], ident[:], ident[:])

        # ---- A chain: exp (bf16 out) -> PE-transpose per (ki, t) -> aT ----
        a_exp = pool.tile([P, NT, K], BF16)
        nc.scalar.activation(a_exp[:], a_raw[:], AF.Exp)

        aT = pool.tile([P, KT, NT, P], BF16)  # [k, (ki, t, n)]
        for ki in range(KT):
            for t in range(NT):
                tp = tpsum.tile([P, P], BF16)
                nc.tensor.transpose(
                    tp[:], a_exp[:, t, ki * P : (ki + 1) * P], ident[:]
                )
                nc.vector.tensor_copy(aT[:, ki, t, :], tp[:])

        # ---- B chain: exp per chunk (bf16 out) ----
        b_exp = pool.tile([P, KT, SM], BF16)
        for c in range(B_CHUNKS):
            nc.scalar.activation(
                b_exp[:, c * KC : (c + 1) * KC, :],
                b_raw[:, c * KC : (c + 1) * KC, :],
                AF.Exp,
            )

        # D2: waits only on DVE (last aT copy); the first real matmul of the
        # accumulation groups then only needs the ACT (b_exp chunk) wait.
        nc.tensor.transpose(dummy[:], aT[:, KT - 1, NT - 1, :], ident[:])

        # ---- matmul: psum[t] += aT[ki,t].T @ b_exp[ki] over ki (ki-outer so
        # chunk-0 matmuls start before b chunk 1 has landed) ----
        out_sb = pool.tile([P, NT, SM], F32)
        for t in range(NT):
            ps = opsum.tile([P, SM], F32)
            for ki in range(KT):
                nc.tensor.matmul(
                    ps[:],
                    aT[:, ki, t, :],
                    b_exp[:, ki, :],
                    start=(ki == 0),
                    stop=(ki == KT - 1),
                )
            nc.scalar.activation(out_sb[:, t, :], ps[:], AF.Ln)
            nc.sync.dma_start(out[t * P : (t + 1) * P, :], out_sb[:, t, :])

    return nc


_NC_CACHE: list = []


def _get_nc() -> bass.Bass:
    if not _NC_CACHE:
        _NC_CACHE.append(_build_nc())
    return _NC_CACHE[0]


def kernel(log_A: np.ndarray, log_B: np.ndarray) -> np.ndarray:
    log_A = np.ascontiguousarray(np.asarray(log_A, dtype=np.float32))
    log_B = np.ascontiguousarray(np.asarray(log_B, dtype=np.float32))
    assert log_A.shape == (N, K) and log_B.shape == (K, M)

    in_maps = []
    for c in range(GRID_N * GRID_M):
        i, j = divmod(c, GRID_M)
        in_maps.append(
            {
                "a_in": np.ascontiguousarray(log_A[i * SN : (i + 1) * SN, :]),
                "b_in": np.ascontiguousarray(log_B[:, j * SM : (j + 1) * SM]),
            }
        )

    nc = _get_nc()
    trace = bool(int(os.environ.get("KERNEL_TRACE", "0")))
    res = run_bass_kernel_spmd(
        nc,
        in_maps,
        list(range(GRID_N * GRID_M)),
        trace=trace,
        tmpdir=globals().get("_TRACE_TMPDIR") if trace else None,
    )

    out = np.empty((N, M), dtype=np.float32)
    for c, r in enumerate(res.results):
        i, j = divmod(c, GRID_M)
        out[i * SN : (i + 1) * SN, j * SM : (j + 1) * SM] = r["out"]
    # stash for test harness introspection
    kernel.last_results = res
    return out
